# revision 10
# baseline (speedup 1.0000x reference)
"""Trainium2 Bass kernel for nn_HausdorffDistance (retrieval_knn).

Computes, for each of B*T = 8 independent problems (sharded 1 problem/core
across 8 NeuronCores):
    nn_dist[i] = min_j ||data1[i] - data2[j]||  (N=M=4096, D=3)
    out[b]     = mean over (t, i) of nn_dist

Device-side algorithm (per core):
  r[i,j] = |b_j|^2 - 2 a_i . b_j   computed on the TensorEngine via a
  split-bf16 matmul (each f32 value split into 3 bf16 terms; K=21 rows),
  accumulated in f32 PSUM.  PSUM evacuation is split between the two
  engines that can read PSUM:
    - DVE tensor_reduce(min) on cols [0, P_DVE)          (1 f32/cycle)
    - ACT activation(Identity, bias=+|a_i|^2, out=bf16) stages cols
      [P_DVE, 2048) to SBUF; the bias makes the values d^2 >= 0 so bf16's
      relative precision is sufficient near the minimum.
  The staged bf16 share is then reduced by a DVE TENSOR_SCALAR
  min-accumulate running in the 4x DVE perf mode (all-SBUF, 2-byte
  packed operands); GPSIMD/Pool supports no min op in this toolchain.
  Host combines partial mins, adds |a_i|^2 to the raw-r share, sqrt, mean.
"""

import sys

sys.path.insert(0, "/opt/trn_rl_repo")

from contextlib import ExitStack

import ml_dtypes
import numpy as np

import concourse.bass as bass
import concourse.tile as tile
from concourse import mybir
from concourse.bass_utils import run_bass_kernel_spmd
from concourse.tile import ScopedClock

BF16 = ml_dtypes.bfloat16

N = 4096          # points per set
K = 21            # split-matmul contraction rows
M_TILES = 32      # 4096 / 128 i-tiles
J_HALF = 2048     # j columns per PSUM chunk (4 banks)
P_DVE = 708       # PSUM cols DVE min-reduces directly
C_ACT = J_HALF - P_DVE   # cols ACT stages PSUM->SBUF bf16 (+a^2 bias)
FMAX = 3.0e38


def _patch_tile_drain():
    """Walrus (CoreV3) rejects the TileContext tail Drain when it carries >1
    sem wait ("Too many sync wait commands").  Split the waits across
    preceding SP NOPs, one wait each."""
    if getattr(tile.TileContext, "_drain_patched", False):
        return

    def _drain_and_barrier(self, tick_clock, wait_clock):
        nc = self.nc
        nops = [nc.sync.nop() for _ in range(31)]
        drain_inst = nc.sync.drain()
        wait_clock.add_sem_waits(
            drain_inst.ins, ScopedClock({None: tick_clock.global_clock})
        )
        si = drain_inst.ins.sync_info
        waits = list(si.on_wait or [])
        if len(waits) > 1:
            si.on_wait = waits[:1]
            for k, w in enumerate(waits[1:]):
                nsi = nops[k].ins.sync_info
                if nsi is None:
                    nops[k].ins.sync_info = mybir.SyncInfo(on_wait=[w], on_update=[])
                else:
                    nsi.on_wait = (nsi.on_wait or []) + [w]
        nc.all_engine_barrier()
        popped = nc._tile_sem_poison_stack.pop()
        assert popped is self._sem_poison
        nc.clear_and_free_semaphores(list(self.sems.allocated().values()))
        nc.all_engine_barrier()

    tile.TileContext._drain_and_barrier = _drain_and_barrier
    tile.TileContext._drain_patched = True


_NC_CACHE = None


def _split_multi_waits(nc):
    """This walrus build allows only 1 sem wait per instruction.  Hoist extra
    waits onto the nearest preceding same-engine instruction with a free wait
    slot (in-order engines: waiting earlier is strictly more conservative);
    if none exists, insert a same-engine NOP carrying the wait just before."""
    eng_map = {
        mybir.EngineType.SP: nc.sync,
        mybir.EngineType.PE: nc.tensor,
        mybir.EngineType.DVE: nc.vector,
        mybir.EngineType.Activation: nc.scalar,
        mybir.EngineType.Pool: nc.gpsimd,
    }
    for bb in nc.m.functions[0].blocks:
        changed = False
        insts = list(bb.instructions)
        out_insts = []
        for inst in insts:
            si = inst.sync_info
            if not si or not si.on_wait or len(si.on_wait) <= 1:
                out_insts.append(inst)
                continue
            waits = list(si.on_wait)
            extra = waits[1:]
            si.on_wait = waits[:1]
            for w in extra:
                placed = False
                for prev in reversed(out_insts):
                    if prev.engine != inst.engine:
                        continue
                    psi = prev.sync_info
                    if psi is None:
                        prev.sync_info = mybir.SyncInfo(on_wait=[w], on_update=[])
                        placed = True
                    elif not psi.on_wait:
                        psi.on_wait = [w]
                        placed = True
                    break
                if not placed:
                    nop = eng_map[inst.engine].nop()
                    nop_inst = nop.ins
                    # nop() appended itself somewhere; remove it from there
                    for b2 in nc.m.functions[0].blocks:
                        if nop_inst in b2.instructions:
                            b2.instructions.remove(nop_inst)
                            break
                    nop_inst.sync_info = mybir.SyncInfo(on_wait=[w], on_update=[])
                    out_insts.append(nop_inst)
                    changed = True
            out_insts.append(inst)
        if changed:
            bb.instructions = out_insts


def _build_nc():
    global _NC_CACHE
    if _NC_CACHE is not None:
        return _NC_CACHE
    _patch_tile_drain()

    nc = bass.Bass(
        "TRN2",
        target_bir_lowering=False,
        debug=False,
        enable_asserts=False,
        num_devices=8,
    )
    inp_ap = nc.dram_tensor("inp", [K, 2 * N], mybir.dt.bfloat16, kind="ExternalInput").ap()
    a2_ap = nc.dram_tensor("a2", [128, M_TILES], mybir.dt.float32, kind="ExternalInput").ap()
    mins_ap = nc.dram_tensor("mins", [128, 4 * M_TILES], mybir.dt.float32, kind="ExternalOutput").ap()

    f32 = mybir.dt.float32
    bf16 = mybir.dt.bfloat16
    amin = mybir.AluOpType.min
    with tile.TileContext(nc) as tc:
        with ExitStack() as ctx:
            consts = ctx.enter_context(tc.tile_pool(name="consts", bufs=1))
            psum = ctx.enter_context(tc.tile_pool(name="psum", bufs=2, space="PSUM"))
            outp = ctx.enter_context(tc.tile_pool(name="outp", bufs=1))
            stage = ctx.enter_context(tc.tile_pool(name="stage", bufs=3))

            inp_sb = consts.tile([K, 2 * N], mybir.dt.bfloat16)
            nc.sync.dma_start(inp_sb[:], inp_ap[:])
            a2_sb = consts.tile([128, M_TILES], f32)
            nc.sync.dma_start(a2_sb[:], a2_ap[:])

            # cols 0..63: DVE r-min (host adds a^2); 64..127: staged d^2 min
            mins_sb = outp.tile([128, 4 * M_TILES], f32)
            trash_b = outp.tile([128, C_ACT], bf16)

            for m in range(M_TILES):
                lw = inp_sb[:, m * 128 : (m + 1) * 128]
                for h in range(2):
                    pt = psum.tile([128, J_HALF], f32)
                    for q in range(4):
                        j0 = N + h * J_HALF + q * 512
                        nc.tensor.matmul(
                            pt[:, q * 512 : (q + 1) * 512],
                            lw,
                            inp_sb[:, j0 : j0 + 512],
                            start=True,
                            stop=True,
                        )
                    col = 2 * m + h
                    st = stage.tile([128, C_ACT], bf16)
                    nc.scalar.activation(
                        st[:],
                        pt[:, P_DVE:J_HALF],
                        mybir.ActivationFunctionType.Identity,
                        bias=a2_sb[:, m : m + 1],
                        scale=1.0,
                    )
                    nc.vector.tensor_reduce(
                        mins_sb[:, col : col + 1],
                        pt[:, 0:P_DVE],
                        axis=mybir.AxisListType.X,
                        op=amin,
                    )
                    nc.vector.tensor_scalar(
                        out=trash_b[:],
                        in0=st[:],
                        scalar1=FMAX,
                        scalar2=None,
                        op0=amin,
                        op1=amin,
                        accum_out=mins_sb[:, 64 + col : 65 + col],
                    )
            nc.sync.dma_start(mins_ap[:], mins_sb[:])

    _split_multi_waits(nc)
    _NC_CACHE = nc
    return nc


def _split3(x):
    """x (f32) -> three bf16 parts whose (f32) sum ~= x to ~2^-27 rel."""
    x = x.astype(np.float32)
    h = x.astype(BF16).astype(np.float32)
    r = x - h
    l = r.astype(BF16).astype(np.float32)
    q = (r - l).astype(BF16).astype(np.float32)
    return h, l, q


def _prep_problem(A, B):
    """Build lhsT [K, N] and rhs [K, N] bf16 rows for r = |b|^2 - 2 a.b."""
    b2 = (B.astype(np.float64) ** 2).sum(1).astype(np.float32)
    b2h, b2l, b2q = _split3(b2)
    ah, al, aq = _split3(A)
    bh, bl, bq = _split3(B)
    ones = np.ones(N, np.float32)
    lhs_rows = [ones, ones, ones]
    rhs_rows = [b2h, b2l, b2q]
    for d in range(3):
        for a_, b_ in (
            (ah[:, d], -2.0 * bh[:, d]),
            (ah[:, d], -2.0 * bl[:, d]),
            (al[:, d], -2.0 * bh[:, d]),
            (al[:, d], -2.0 * bl[:, d]),
            (ah[:, d], -2.0 * bq[:, d]),
            (aq[:, d], -2.0 * bh[:, d]),
        ):
            lhs_rows.append(a_)
            rhs_rows.append(b_)
    lhsT = np.stack(lhs_rows).astype(BF16)
    rhs = np.stack(rhs_rows).astype(BF16)
    return np.concatenate([lhsT, rhs], axis=1)  # [K, 2N]


def _run(data1, data2, trace=False):
    d1 = np.asarray(data1, dtype=np.float32).reshape(8, N, 3)
    d2 = np.asarray(data2, dtype=np.float32).reshape(8, N, 3)
    in_maps = []
    a2_all = []
    for p in range(8):
        a2 = (d1[p].astype(np.float64) ** 2).sum(1).astype(np.float32)
        a2_all.append(a2)
        in_maps.append(
            {
                "inp": _prep_problem(d1[p], d2[p]),
                "a2": np.ascontiguousarray(a2.reshape(M_TILES, 128).T),
            }
        )
    nc = _build_nc()
    res = run_bass_kernel_spmd(nc, in_maps, core_ids=list(range(8)), trace=trace)

    out = np.zeros(2, np.float64)
    for p in range(8):
        m = res.results[p]["mins"]          # [128, 128]
        a2c = in_maps[p]["a2"]              # [128, 32]
        mr = m[:, :64].reshape(128, M_TILES, 2).min(axis=-1) + a2c   # DVE share
        ms = m[:, 64:].reshape(128, M_TILES, 2).min(axis=-1)         # staged share
        mflat = np.minimum(mr, ms).T.reshape(N).astype(np.float64)
        dd = np.sqrt(np.maximum(mflat, 0.0))
        out[p // 4] += dd.mean() / 4.0
    return out.astype(np.float32), res


def kernel(data1, data2, dim):
    dim = int(dim)
    if dim > 0:
        data1 = np.swapaxes(np.asarray(data1), 0, dim)
        data2 = np.swapaxes(np.asarray(data2), 0, dim)
    out, _ = _run(data1, data2, trace=False)
    return out


def kernel_traced(data1, data2, dim):
    """test.py entry: returns (output, BassKernelResults) with profiling."""
    dim = int(dim)
    if dim > 0:
        data1 = np.swapaxes(np.asarray(data1), 0, dim)
        data2 = np.swapaxes(np.asarray(data2), 0, dim)
    return _run(data1, data2, trace=True)


# revision 12
# speedup vs baseline: 1.2984x; 1.2984x over previous
"""Trainium2 Bass kernel for nn_HausdorffDistance (retrieval_knn).

Computes, for each of B*T = 8 independent problems (sharded 1 problem/core
across 8 NeuronCores):
    nn_dist[i] = min_j ||data1[i] - data2[j]||  (N=M=4096, D=3)
    out[b]     = mean over (t, i) of nn_dist

Device-side algorithm (per core):
  r[i,j] = |b_j|^2 - 2 a_i . b_j   computed on the TensorEngine via a
  split-bf16 matmul (each f32 value split into 3 bf16 terms; K=21 rows),
  accumulated in f32 PSUM.  PSUM evacuation is split between the two
  engines that can read PSUM:
    - DVE tensor_reduce(min) on cols [0, P_DVE)          (1 f32/cycle)
    - ACT activation(Identity, bias=+|a_i|^2, out=bf16) stages cols
      [P_DVE, 2048) to SBUF; the bias makes the values d^2 >= 0 so bf16's
      relative precision is sufficient near the minimum.
  The staged bf16 share is then reduced by a DVE TENSOR_SCALAR
  min-accumulate running in the 4x DVE perf mode (all-SBUF, 2-byte
  packed operands); GPSIMD/Pool supports no min op in this toolchain.
  Host combines partial mins, adds |a_i|^2 to the raw-r share, sqrt, mean.
"""

import sys

sys.path.insert(0, "/opt/trn_rl_repo")

from contextlib import ExitStack

import ml_dtypes
import numpy as np

import concourse.bass as bass
import concourse.tile as tile
from concourse import mybir
from concourse.bass_utils import run_bass_kernel_spmd
from concourse.tile import ScopedClock

BF16 = ml_dtypes.bfloat16

N = 4096          # points per set
K = 21            # split-matmul contraction rows
M_TILES = 32      # 4096 / 128 i-tiles
J_HALF = 2048     # j columns per PSUM chunk (4 banks)
# Alternating per-chunk split (PSUM banks are 512 f32 wide; separate tiles
# per reader avoid the tile scheduler's same-tile cross-engine ordering):
# even chunks 512 DVE / 1536 ACT, odd chunks 1024 DVE / 1024 ACT.
SPLITS = ((512, 1536), (1024, 1024))
FMAX = 3.0e38


def _patch_tile_drain():
    """Walrus (CoreV3) rejects the TileContext tail Drain when it carries >1
    sem wait ("Too many sync wait commands").  Split the waits across
    preceding SP NOPs, one wait each."""
    if getattr(tile.TileContext, "_drain_patched", False):
        return

    def _drain_and_barrier(self, tick_clock, wait_clock):
        nc = self.nc
        nops = [nc.sync.nop() for _ in range(31)]
        drain_inst = nc.sync.drain()
        wait_clock.add_sem_waits(
            drain_inst.ins, ScopedClock({None: tick_clock.global_clock})
        )
        si = drain_inst.ins.sync_info
        waits = list(si.on_wait or [])
        if len(waits) > 1:
            si.on_wait = waits[:1]
            for k, w in enumerate(waits[1:]):
                nsi = nops[k].ins.sync_info
                if nsi is None:
                    nops[k].ins.sync_info = mybir.SyncInfo(on_wait=[w], on_update=[])
                else:
                    nsi.on_wait = (nsi.on_wait or []) + [w]
        nc.all_engine_barrier()
        popped = nc._tile_sem_poison_stack.pop()
        assert popped is self._sem_poison
        nc.clear_and_free_semaphores(list(self.sems.allocated().values()))
        nc.all_engine_barrier()

    tile.TileContext._drain_and_barrier = _drain_and_barrier
    tile.TileContext._drain_patched = True


_NC_CACHE = None


def _split_multi_waits(nc):
    """This walrus build allows only 1 sem wait per instruction.  Hoist extra
    waits onto the nearest preceding same-engine instruction with a free wait
    slot (in-order engines: waiting earlier is strictly more conservative);
    if none exists, insert a same-engine NOP carrying the wait just before."""
    eng_map = {
        mybir.EngineType.SP: nc.sync,
        mybir.EngineType.PE: nc.tensor,
        mybir.EngineType.DVE: nc.vector,
        mybir.EngineType.Activation: nc.scalar,
        mybir.EngineType.Pool: nc.gpsimd,
    }
    for bb in nc.m.functions[0].blocks:
        changed = False
        insts = list(bb.instructions)
        out_insts = []
        for inst in insts:
            si = inst.sync_info
            if not si or not si.on_wait or len(si.on_wait) <= 1:
                out_insts.append(inst)
                continue
            waits = list(si.on_wait)
            extra = waits[1:]
            si.on_wait = waits[:1]
            for w in extra:
                placed = False
                for prev in reversed(out_insts):
                    if prev.engine != inst.engine:
                        continue
                    psi = prev.sync_info
                    if psi is None:
                        prev.sync_info = mybir.SyncInfo(on_wait=[w], on_update=[])
                        placed = True
                    elif not psi.on_wait:
                        psi.on_wait = [w]
                        placed = True
                    break
                if not placed:
                    nop = eng_map[inst.engine].nop()
                    nop_inst = nop.ins
                    # nop() appended itself somewhere; remove it from there
                    for b2 in nc.m.functions[0].blocks:
                        if nop_inst in b2.instructions:
                            b2.instructions.remove(nop_inst)
                            break
                    nop_inst.sync_info = mybir.SyncInfo(on_wait=[w], on_update=[])
                    out_insts.append(nop_inst)
                    changed = True
            out_insts.append(inst)
        if changed:
            bb.instructions = out_insts


def _build_nc():
    global _NC_CACHE
    if _NC_CACHE is not None:
        return _NC_CACHE
    _patch_tile_drain()

    nc = bass.Bass(
        "TRN2",
        target_bir_lowering=False,
        debug=False,
        enable_asserts=False,
        num_devices=8,
    )
    inp_ap = nc.dram_tensor("inp", [K, 2 * N], mybir.dt.bfloat16, kind="ExternalInput").ap()
    a2_ap = nc.dram_tensor("a2", [128, M_TILES], mybir.dt.float32, kind="ExternalInput").ap()
    mins_ap = nc.dram_tensor("mins", [128, 4 * M_TILES], mybir.dt.float32, kind="ExternalOutput").ap()

    f32 = mybir.dt.float32
    bf16 = mybir.dt.bfloat16
    amin = mybir.AluOpType.min
    with tile.TileContext(nc) as tc:
        with ExitStack() as ctx:
            consts = ctx.enter_context(tc.tile_pool(name="consts", bufs=1))
            # One PSUM pool per (phase, reader): single-reader tiles so the
            # tile scheduler never orders DVE and ACT against each other.
            psumR = [
                ctx.enter_context(
                    tc.tile_pool(name=f"psumR{i}", bufs=1, space="PSUM")
                )
                for i in range(2)
            ]
            psumS = [
                ctx.enter_context(
                    tc.tile_pool(name=f"psumS{i}", bufs=1, space="PSUM")
                )
                for i in range(2)
            ]
            outp = ctx.enter_context(tc.tile_pool(name="outp", bufs=1))
            stage = ctx.enter_context(tc.tile_pool(name="stage", bufs=3))

            inp_sb = consts.tile([K, 2 * N], mybir.dt.bfloat16)
            nc.sync.dma_start(inp_sb[:], inp_ap[:])
            a2_sb = consts.tile([128, M_TILES], f32)
            nc.sync.dma_start(a2_sb[:], a2_ap[:])

            # cols 0..63: DVE r-min (host adds a^2); 64..127: staged d^2 min
            mins_sb = outp.tile([128, 4 * M_TILES], f32)
            trash_b = outp.tile([128, max(s for _, s in SPLITS)], bf16)

            for m in range(M_TILES):
                lw = inp_sb[:, m * 128 : (m + 1) * 128]
                for h in range(2):
                    col = 2 * m + h
                    phase = col % 2
                    r_w, s_w = SPLITS[phase]
                    ptR = psumR[phase].tile([128, r_w], f32)
                    ptS = psumS[phase].tile([128, s_w], f32)
                    j0 = N + h * J_HALF
                    for q in range(r_w // 512):
                        nc.tensor.matmul(
                            ptR[:, q * 512 : (q + 1) * 512],
                            lw,
                            inp_sb[:, j0 + q * 512 : j0 + (q + 1) * 512],
                            start=True,
                            stop=True,
                        )
                    for q in range(s_w // 512):
                        nc.tensor.matmul(
                            ptS[:, q * 512 : (q + 1) * 512],
                            lw,
                            inp_sb[:, j0 + r_w + q * 512 : j0 + r_w + (q + 1) * 512],
                            start=True,
                            stop=True,
                        )
                    st = stage.tile([128, s_w], bf16)
                    nc.scalar.activation(
                        st[:],
                        ptS[:],
                        mybir.ActivationFunctionType.Identity,
                        bias=a2_sb[:, m : m + 1],
                        scale=1.0,
                    )
                    nc.vector.tensor_reduce(
                        mins_sb[:, col : col + 1],
                        ptR[:],
                        axis=mybir.AxisListType.X,
                        op=amin,
                    )
                    nc.vector.tensor_scalar(
                        out=trash_b[:, 0:s_w],
                        in0=st[:],
                        scalar1=FMAX,
                        scalar2=None,
                        op0=amin,
                        op1=amin,
                        accum_out=mins_sb[:, 64 + col : 65 + col],
                    )
            nc.sync.dma_start(mins_ap[:], mins_sb[:])

    _split_multi_waits(nc)
    _NC_CACHE = nc
    return nc


def _split3(x):
    """x (f32) -> three bf16 parts whose (f32) sum ~= x to ~2^-27 rel."""
    x = x.astype(np.float32)
    h = x.astype(BF16).astype(np.float32)
    r = x - h
    l = r.astype(BF16).astype(np.float32)
    q = (r - l).astype(BF16).astype(np.float32)
    return h, l, q


def _prep_problem(A, B):
    """Build lhsT [K, N] and rhs [K, N] bf16 rows for r = |b|^2 - 2 a.b."""
    b2 = (B.astype(np.float64) ** 2).sum(1).astype(np.float32)
    b2h, b2l, b2q = _split3(b2)
    ah, al, aq = _split3(A)
    bh, bl, bq = _split3(B)
    ones = np.ones(N, np.float32)
    lhs_rows = [ones, ones, ones]
    rhs_rows = [b2h, b2l, b2q]
    for d in range(3):
        for a_, b_ in (
            (ah[:, d], -2.0 * bh[:, d]),
            (ah[:, d], -2.0 * bl[:, d]),
            (al[:, d], -2.0 * bh[:, d]),
            (al[:, d], -2.0 * bl[:, d]),
            (ah[:, d], -2.0 * bq[:, d]),
            (aq[:, d], -2.0 * bh[:, d]),
        ):
            lhs_rows.append(a_)
            rhs_rows.append(b_)
    lhsT = np.stack(lhs_rows).astype(BF16)
    rhs = np.stack(rhs_rows).astype(BF16)
    return np.concatenate([lhsT, rhs], axis=1)  # [K, 2N]


def _run(data1, data2, trace=False):
    d1 = np.asarray(data1, dtype=np.float32).reshape(8, N, 3)
    d2 = np.asarray(data2, dtype=np.float32).reshape(8, N, 3)
    in_maps = []
    a2_all = []
    for p in range(8):
        a2 = (d1[p].astype(np.float64) ** 2).sum(1).astype(np.float32)
        a2_all.append(a2)
        in_maps.append(
            {
                "inp": _prep_problem(d1[p], d2[p]),
                "a2": np.ascontiguousarray(a2.reshape(M_TILES, 128).T),
            }
        )
    nc = _build_nc()
    res = run_bass_kernel_spmd(nc, in_maps, core_ids=list(range(8)), trace=trace)

    out = np.zeros(2, np.float64)
    for p in range(8):
        m = res.results[p]["mins"]          # [128, 128]
        a2c = in_maps[p]["a2"]              # [128, 32]
        mr = m[:, :64].reshape(128, M_TILES, 2).min(axis=-1) + a2c   # DVE share
        ms = m[:, 64:].reshape(128, M_TILES, 2).min(axis=-1)         # staged share
        mflat = np.minimum(mr, ms).T.reshape(N).astype(np.float64)
        dd = np.sqrt(np.maximum(mflat, 0.0))
        out[p // 4] += dd.mean() / 4.0
    return out.astype(np.float32), res


def kernel(data1, data2, dim):
    dim = int(dim)
    if dim > 0:
        data1 = np.swapaxes(np.asarray(data1), 0, dim)
        data2 = np.swapaxes(np.asarray(data2), 0, dim)
    out, _ = _run(data1, data2, trace=False)
    return out


def kernel_traced(data1, data2, dim):
    """test.py entry: returns (output, BassKernelResults) with profiling."""
    dim = int(dim)
    if dim > 0:
        data1 = np.swapaxes(np.asarray(data1), 0, dim)
        data2 = np.swapaxes(np.asarray(data2), 0, dim)
    return _run(data1, data2, trace=True)


# revision 18
# speedup vs baseline: 1.3407x; 1.0326x over previous
"""Trainium2 Bass kernel for nn_HausdorffDistance (retrieval_knn).

Computes, for each of B*T = 8 independent problems (sharded 1 problem/core
across 8 NeuronCores):
    nn_dist[i] = min_j ||data1[i] - data2[j]||  (N=M=4096, D=3)
    out[b]     = mean over (t, i) of nn_dist

Device-side algorithm (per core):
  r[i,j] = |b_j|^2 - 2 a_i . b_j   computed on the TensorEngine via a
  split-bf16 matmul (each f32 value split into 3 bf16 terms; K=21 rows),
  accumulated in f32 PSUM.  PSUM evacuation is split between the two
  engines that can read PSUM:
    - DVE tensor_reduce(min) on cols [0, P_DVE)          (1 f32/cycle)
    - ACT activation(Identity, bias=+|a_i|^2, out=bf16) stages cols
      [P_DVE, 2048) to SBUF; the bias makes the values d^2 >= 0 so bf16's
      relative precision is sufficient near the minimum.
  The staged bf16 share is then reduced by a DVE TENSOR_SCALAR
  min-accumulate running in the 4x DVE perf mode (all-SBUF, 2-byte
  packed operands); GPSIMD/Pool supports no min op in this toolchain.
  Host combines partial mins, adds |a_i|^2 to the raw-r share, sqrt, mean.
"""

import sys

sys.path.insert(0, "/opt/trn_rl_repo")

from contextlib import ExitStack

import ml_dtypes
import numpy as np

import concourse.bass as bass
import concourse.tile as tile
from concourse import mybir
from concourse.bass_utils import run_bass_kernel_spmd
from concourse.tile import ScopedClock

BF16 = ml_dtypes.bfloat16

N = 4096          # points per set
K = 21            # split-matmul contraction rows
M_TILES = 32      # 4096 / 128 i-tiles
J_HALF = 2048     # j columns per PSUM chunk (4 banks)
# Alternating per-chunk split (PSUM banks are 512 f32 wide; separate tiles
# per reader avoid the tile scheduler's same-tile cross-engine ordering):
# even chunks 512 DVE / 1536 ACT, odd chunks 1024 DVE / 1024 ACT.
SPLITS = ((512, 1536), (1024, 1024))
FMAX = 3.0e38


def _patch_tile_drain():
    """Walrus (CoreV3) rejects the TileContext tail Drain when it carries >1
    sem wait ("Too many sync wait commands").  Split the waits across
    preceding SP NOPs, one wait each."""
    if getattr(tile.TileContext, "_drain_patched", False):
        return

    def _drain_and_barrier(self, tick_clock, wait_clock):
        nc = self.nc
        nops = [nc.sync.nop() for _ in range(31)]
        drain_inst = nc.sync.drain()
        wait_clock.add_sem_waits(
            drain_inst.ins, ScopedClock({None: tick_clock.global_clock})
        )
        si = drain_inst.ins.sync_info
        waits = list(si.on_wait or [])
        if len(waits) > 1:
            si.on_wait = waits[:1]
            for k, w in enumerate(waits[1:]):
                nsi = nops[k].ins.sync_info
                if nsi is None:
                    nops[k].ins.sync_info = mybir.SyncInfo(on_wait=[w], on_update=[])
                else:
                    nsi.on_wait = (nsi.on_wait or []) + [w]
        nc.all_engine_barrier()
        popped = nc._tile_sem_poison_stack.pop()
        assert popped is self._sem_poison
        nc.clear_and_free_semaphores(list(self.sems.allocated().values()))
        nc.all_engine_barrier()

    tile.TileContext._drain_and_barrier = _drain_and_barrier
    tile.TileContext._drain_patched = True


_NC_CACHE = None


def _split_multi_waits(nc):
    """This walrus build allows only 1 sem wait per instruction.  Hoist extra
    waits onto the nearest preceding same-engine instruction with a free wait
    slot (in-order engines: waiting earlier is strictly more conservative);
    if none exists, insert a same-engine NOP carrying the wait just before."""
    eng_map = {
        mybir.EngineType.SP: nc.sync,
        mybir.EngineType.PE: nc.tensor,
        mybir.EngineType.DVE: nc.vector,
        mybir.EngineType.Activation: nc.scalar,
        mybir.EngineType.Pool: nc.gpsimd,
    }
    for bb in nc.m.functions[0].blocks:
        changed = False
        insts = list(bb.instructions)
        out_insts = []
        for inst in insts:
            si = inst.sync_info
            if not si or not si.on_wait or len(si.on_wait) <= 1:
                out_insts.append(inst)
                continue
            waits = list(si.on_wait)
            extra = waits[1:]
            si.on_wait = waits[:1]
            for w in extra:
                placed = False
                for prev in reversed(out_insts):
                    if prev.engine != inst.engine:
                        continue
                    psi = prev.sync_info
                    if psi is None:
                        prev.sync_info = mybir.SyncInfo(on_wait=[w], on_update=[])
                        placed = True
                    elif not psi.on_wait:
                        psi.on_wait = [w]
                        placed = True
                    break
                if not placed:
                    nop = eng_map[inst.engine].nop()
                    nop_inst = nop.ins
                    # nop() appended itself somewhere; remove it from there
                    for b2 in nc.m.functions[0].blocks:
                        if nop_inst in b2.instructions:
                            b2.instructions.remove(nop_inst)
                            break
                    nop_inst.sync_info = mybir.SyncInfo(on_wait=[w], on_update=[])
                    out_insts.append(nop_inst)
                    changed = True
            out_insts.append(inst)
        if changed:
            bb.instructions = out_insts


def _build_nc():
    global _NC_CACHE
    if _NC_CACHE is not None:
        return _NC_CACHE
    _patch_tile_drain()

    nc = bass.Bass(
        "TRN2",
        target_bir_lowering=False,
        debug=False,
        enable_asserts=False,
        num_devices=8,
    )
    inp_ap = nc.dram_tensor("inp", [K, 2 * N], mybir.dt.bfloat16, kind="ExternalInput").ap()
    a2_ap = nc.dram_tensor("a2", [128, M_TILES], mybir.dt.float32, kind="ExternalInput").ap()
    mins_ap = nc.dram_tensor("mins", [128, 4 * M_TILES], mybir.dt.float32, kind="ExternalOutput").ap()

    f32 = mybir.dt.float32
    bf16 = mybir.dt.bfloat16
    amin = mybir.AluOpType.min
    with tile.TileContext(nc) as tc:
        with ExitStack() as ctx:
            consts = ctx.enter_context(tc.tile_pool(name="consts", bufs=1))
            # One PSUM pool per (phase, reader): single-reader tiles so the
            # tile scheduler never orders DVE and ACT against each other.
            psumR = [
                ctx.enter_context(
                    tc.tile_pool(name=f"psumR{i}", bufs=1, space="PSUM")
                )
                for i in range(2)
            ]
            psumS = [
                ctx.enter_context(
                    tc.tile_pool(name=f"psumS{i}", bufs=1, space="PSUM")
                )
                for i in range(2)
            ]
            outp = ctx.enter_context(tc.tile_pool(name="outp", bufs=1))
            stage = ctx.enter_context(tc.tile_pool(name="stage", bufs=4))

            inp_sb = consts.tile([K, 2 * N], mybir.dt.bfloat16)
            # Split the input DMA so the first chunks' operands land first.
            nc.sync.dma_start(inp_sb[:, 0:6144], inp_ap[:, 0:6144])
            nc.sync.dma_start(inp_sb[:, 6144:8192], inp_ap[:, 6144:8192])
            a2_sb = consts.tile([128, M_TILES], f32)
            nc.sync.dma_start(a2_sb[:], a2_ap[:])

            # PE p-state warmup: dummy matmuls on a memset scratch keep the
            # PE continuously busy through the slow-ramp window so the real
            # matmuls run at full clock.  They write the same PSUM slot the
            # first S-tile will use; the real matmuls overwrite in-order.
            wsrc = consts.tile([K, 512], mybir.dt.bfloat16)
            nc.gpsimd.memset(wsrc[:], 0)
            warm = psumS[0].tile([128, SPLITS[0][1]], f32, name="ptS", tag="ptS")
            for _ in range(10):
                nc.tensor.matmul(
                    warm[:, 0:512], wsrc[:, 0:128], wsrc[:, 0:512], start=True, stop=True
                )
            warm_tile = warm

            # cols 0..63: DVE r-min (host adds a^2); 64..127: staged d^2 min
            mins_sb = outp.tile([128, 4 * M_TILES], f32)
            trash_b = outp.tile([128, max(s for _, s in SPLITS)], bf16)

            for m in range(M_TILES):
                lw = inp_sb[:, m * 128 : (m + 1) * 128]
                for h in range(2):
                    col = 2 * m + h
                    phase = col % 2
                    r_w, s_w = SPLITS[phase]
                    ptR = psumR[phase].tile([128, r_w], f32)
                    if warm_tile is not None and phase == 0:
                        ptS, warm_tile = warm_tile, None
                    else:
                        ptS = psumS[phase].tile([128, s_w], f32, name="ptS", tag="ptS")
                    j0 = N + h * J_HALF
                    for q in range(r_w // 512):
                        nc.tensor.matmul(
                            ptR[:, q * 512 : (q + 1) * 512],
                            lw,
                            inp_sb[:, j0 + q * 512 : j0 + (q + 1) * 512],
                            start=True,
                            stop=True,
                        )
                    for q in range(s_w // 512):
                        nc.tensor.matmul(
                            ptS[:, q * 512 : (q + 1) * 512],
                            lw,
                            inp_sb[:, j0 + r_w + q * 512 : j0 + r_w + (q + 1) * 512],
                            start=True,
                            stop=True,
                        )
                    st = stage.tile([128, s_w], bf16)
                    nc.scalar.activation(
                        st[:],
                        ptS[:],
                        mybir.ActivationFunctionType.Identity,
                        bias=a2_sb[:, m : m + 1],
                        scale=1.0,
                    )
                    nc.vector.tensor_reduce(
                        mins_sb[:, col : col + 1],
                        ptR[:],
                        axis=mybir.AxisListType.X,
                        op=amin,
                    )
                    nc.vector.tensor_scalar(
                        out=trash_b[:, 0:s_w],
                        in0=st[:],
                        scalar1=FMAX,
                        scalar2=None,
                        op0=amin,
                        op1=amin,
                        accum_out=mins_sb[:, 64 + col : 65 + col],
                    )
                    if col == 47:
                        # Early partial output DMA overlaps the write-back
                        # latency with the remaining chunks.
                        nc.sync.dma_start(mins_ap[:, 0:48], mins_sb[:, 0:48])
                        nc.sync.dma_start(mins_ap[:, 64:112], mins_sb[:, 64:112])
            nc.sync.dma_start(mins_ap[:, 48:64], mins_sb[:, 48:64])
            nc.sync.dma_start(mins_ap[:, 112:128], mins_sb[:, 112:128])

    _split_multi_waits(nc)
    _NC_CACHE = nc
    return nc


def _split3(x):
    """x (f32) -> three bf16 parts whose (f32) sum ~= x to ~2^-27 rel."""
    x = x.astype(np.float32)
    h = x.astype(BF16).astype(np.float32)
    r = x - h
    l = r.astype(BF16).astype(np.float32)
    q = (r - l).astype(BF16).astype(np.float32)
    return h, l, q


def _prep_problem(A, B):
    """Build lhsT [K, N] and rhs [K, N] bf16 rows for r = |b|^2 - 2 a.b."""
    b2 = (B.astype(np.float64) ** 2).sum(1).astype(np.float32)
    b2h, b2l, b2q = _split3(b2)
    ah, al, aq = _split3(A)
    bh, bl, bq = _split3(B)
    ones = np.ones(N, np.float32)
    lhs_rows = [ones, ones, ones]
    rhs_rows = [b2h, b2l, b2q]
    for d in range(3):
        for a_, b_ in (
            (ah[:, d], -2.0 * bh[:, d]),
            (ah[:, d], -2.0 * bl[:, d]),
            (al[:, d], -2.0 * bh[:, d]),
            (al[:, d], -2.0 * bl[:, d]),
            (ah[:, d], -2.0 * bq[:, d]),
            (aq[:, d], -2.0 * bh[:, d]),
        ):
            lhs_rows.append(a_)
            rhs_rows.append(b_)
    lhsT = np.stack(lhs_rows).astype(BF16)
    rhs = np.stack(rhs_rows).astype(BF16)
    return np.concatenate([lhsT, rhs], axis=1)  # [K, 2N]


def _run(data1, data2, trace=False):
    d1 = np.asarray(data1, dtype=np.float32).reshape(8, N, 3)
    d2 = np.asarray(data2, dtype=np.float32).reshape(8, N, 3)
    in_maps = []
    a2_all = []
    for p in range(8):
        a2 = (d1[p].astype(np.float64) ** 2).sum(1).astype(np.float32)
        a2_all.append(a2)
        in_maps.append(
            {
                "inp": _prep_problem(d1[p], d2[p]),
                "a2": np.ascontiguousarray(a2.reshape(M_TILES, 128).T),
            }
        )
    nc = _build_nc()
    res = run_bass_kernel_spmd(nc, in_maps, core_ids=list(range(8)), trace=trace)

    out = np.zeros(2, np.float64)
    for p in range(8):
        m = res.results[p]["mins"]          # [128, 128]
        a2c = in_maps[p]["a2"]              # [128, 32]
        mr = m[:, :64].reshape(128, M_TILES, 2).min(axis=-1) + a2c   # DVE share
        ms = m[:, 64:].reshape(128, M_TILES, 2).min(axis=-1)         # staged share
        mflat = np.minimum(mr, ms).T.reshape(N).astype(np.float64)
        dd = np.sqrt(np.maximum(mflat, 0.0))
        out[p // 4] += dd.mean() / 4.0
    return out.astype(np.float32), res


def kernel(data1, data2, dim):
    dim = int(dim)
    if dim > 0:
        data1 = np.swapaxes(np.asarray(data1), 0, dim)
        data2 = np.swapaxes(np.asarray(data2), 0, dim)
    out, _ = _run(data1, data2, trace=False)
    return out


def kernel_traced(data1, data2, dim):
    """test.py entry: returns (output, BassKernelResults) with profiling."""
    dim = int(dim)
    if dim > 0:
        data1 = np.swapaxes(np.asarray(data1), 0, dim)
        data2 = np.swapaxes(np.asarray(data2), 0, dim)
    return _run(data1, data2, trace=True)


# revision 19
# speedup vs baseline: 1.3669x; 1.0196x over previous
"""Trainium2 Bass kernel for nn_HausdorffDistance (retrieval_knn).

Computes, for each of B*T = 8 independent problems (sharded 1 problem/core
across 8 NeuronCores):
    nn_dist[i] = min_j ||data1[i] - data2[j]||  (N=M=4096, D=3)
    out[b]     = mean over (t, i) of nn_dist

Device-side algorithm (per core):
  r[i,j] = |b_j|^2 - 2 a_i . b_j   computed on the TensorEngine via a
  split-bf16 matmul (each f32 value split into 3 bf16 terms; K=21 rows),
  accumulated in f32 PSUM.  PSUM evacuation is split between the two
  engines that can read PSUM:
    - DVE tensor_reduce(min) on cols [0, P_DVE)          (1 f32/cycle)
    - ACT activation(Identity, bias=+|a_i|^2, out=bf16) stages cols
      [P_DVE, 2048) to SBUF; the bias makes the values d^2 >= 0 so bf16's
      relative precision is sufficient near the minimum.
  The staged bf16 share is then reduced by a DVE TENSOR_SCALAR
  min-accumulate running in the 4x DVE perf mode (all-SBUF, 2-byte
  packed operands); GPSIMD/Pool supports no min op in this toolchain.
  Host combines partial mins, adds |a_i|^2 to the raw-r share, sqrt, mean.
"""

import sys

sys.path.insert(0, "/opt/trn_rl_repo")

from contextlib import ExitStack

import ml_dtypes
import numpy as np

import concourse.bass as bass
import concourse.tile as tile
from concourse import mybir
from concourse.bass_utils import run_bass_kernel_spmd
from concourse.tile import ScopedClock

BF16 = ml_dtypes.bfloat16

N = 4096          # points per set
K = 21            # split-matmul contraction rows
M_TILES = 32      # 4096 / 128 i-tiles
J_HALF = 2048     # j columns per PSUM chunk (4 banks)
# Alternating per-chunk split (PSUM banks are 512 f32 wide; separate tiles
# per reader avoid the tile scheduler's same-tile cross-engine ordering):
# even chunks 512 DVE / 1536 ACT, odd chunks 1024 DVE / 1024 ACT.
SPLITS = ((512, 1536), (1024, 1024))
FMAX = 3.0e38


def _patch_tile_drain():
    """Walrus (CoreV3) rejects the TileContext tail Drain when it carries >1
    sem wait ("Too many sync wait commands").  Split the waits across
    preceding SP NOPs, one wait each."""
    if getattr(tile.TileContext, "_drain_patched", False):
        return

    def _drain_and_barrier(self, tick_clock, wait_clock):
        nc = self.nc
        nops = [nc.sync.nop() for _ in range(31)]
        drain_inst = nc.sync.drain()
        wait_clock.add_sem_waits(
            drain_inst.ins, ScopedClock({None: tick_clock.global_clock})
        )
        si = drain_inst.ins.sync_info
        waits = list(si.on_wait or [])
        if len(waits) > 1:
            si.on_wait = waits[:1]
            for k, w in enumerate(waits[1:]):
                nsi = nops[k].ins.sync_info
                if nsi is None:
                    nops[k].ins.sync_info = mybir.SyncInfo(on_wait=[w], on_update=[])
                else:
                    nsi.on_wait = (nsi.on_wait or []) + [w]
        nc.all_engine_barrier()
        popped = nc._tile_sem_poison_stack.pop()
        assert popped is self._sem_poison
        nc.clear_and_free_semaphores(list(self.sems.allocated().values()))
        nc.all_engine_barrier()

    tile.TileContext._drain_and_barrier = _drain_and_barrier
    tile.TileContext._drain_patched = True


_NC_CACHE = None


def _split_multi_waits(nc):
    """This walrus build allows only 1 sem wait per instruction.  Hoist extra
    waits onto the nearest preceding same-engine instruction with a free wait
    slot (in-order engines: waiting earlier is strictly more conservative);
    if none exists, insert a same-engine NOP carrying the wait just before."""
    eng_map = {
        mybir.EngineType.SP: nc.sync,
        mybir.EngineType.PE: nc.tensor,
        mybir.EngineType.DVE: nc.vector,
        mybir.EngineType.Activation: nc.scalar,
        mybir.EngineType.Pool: nc.gpsimd,
    }
    for bb in nc.m.functions[0].blocks:
        changed = False
        insts = list(bb.instructions)
        out_insts = []
        for inst in insts:
            si = inst.sync_info
            if not si or not si.on_wait or len(si.on_wait) <= 1:
                out_insts.append(inst)
                continue
            waits = list(si.on_wait)
            extra = waits[1:]
            si.on_wait = waits[:1]
            for w in extra:
                placed = False
                for prev in reversed(out_insts):
                    if prev.engine != inst.engine:
                        continue
                    psi = prev.sync_info
                    if psi is None:
                        prev.sync_info = mybir.SyncInfo(on_wait=[w], on_update=[])
                        placed = True
                    elif not psi.on_wait:
                        psi.on_wait = [w]
                        placed = True
                    break
                if not placed:
                    nop = eng_map[inst.engine].nop()
                    nop_inst = nop.ins
                    # nop() appended itself somewhere; remove it from there
                    for b2 in nc.m.functions[0].blocks:
                        if nop_inst in b2.instructions:
                            b2.instructions.remove(nop_inst)
                            break
                    nop_inst.sync_info = mybir.SyncInfo(on_wait=[w], on_update=[])
                    out_insts.append(nop_inst)
                    changed = True
            out_insts.append(inst)
        if changed:
            bb.instructions = out_insts


def _build_nc():
    global _NC_CACHE
    if _NC_CACHE is not None:
        return _NC_CACHE
    _patch_tile_drain()

    nc = bass.Bass(
        "TRN2",
        target_bir_lowering=False,
        debug=False,
        enable_asserts=False,
        num_devices=8,
    )
    inp_ap = nc.dram_tensor("inp", [K, 2 * N], mybir.dt.bfloat16, kind="ExternalInput").ap()
    a2_ap = nc.dram_tensor("a2", [128, M_TILES], mybir.dt.float32, kind="ExternalInput").ap()
    mins_ap = nc.dram_tensor("mins", [128, 4 * M_TILES], mybir.dt.float32, kind="ExternalOutput").ap()

    f32 = mybir.dt.float32
    bf16 = mybir.dt.bfloat16
    amin = mybir.AluOpType.min
    with tile.TileContext(nc) as tc:
        with ExitStack() as ctx:
            consts = ctx.enter_context(tc.tile_pool(name="consts", bufs=1))
            # One PSUM pool per (phase, reader): single-reader tiles so the
            # tile scheduler never orders DVE and ACT against each other.
            psumR = [
                ctx.enter_context(
                    tc.tile_pool(name=f"psumR{i}", bufs=1, space="PSUM")
                )
                for i in range(2)
            ]
            psumS = [
                ctx.enter_context(
                    tc.tile_pool(name=f"psumS{i}", bufs=1, space="PSUM")
                )
                for i in range(2)
            ]
            outp = ctx.enter_context(tc.tile_pool(name="outp", bufs=1))
            stage = ctx.enter_context(tc.tile_pool(name="stage", bufs=4))

            inp_sb = consts.tile([K, 2 * N], mybir.dt.bfloat16)
            # Split the input DMA so the first chunks' operands land first.
            nc.sync.dma_start(inp_sb[:, 0:6144], inp_ap[:, 0:6144])
            nc.sync.dma_start(inp_sb[:, 6144:8192], inp_ap[:, 6144:8192])
            a2_sb = consts.tile([128, M_TILES], f32)
            nc.sync.dma_start(a2_sb[:], a2_ap[:])

            # PE p-state warmup: dummy matmuls on a memset scratch keep the
            # PE continuously busy through the slow-ramp window so the real
            # matmuls run at full clock.  They write the same PSUM slot the
            # first S-tile will use; the real matmuls overwrite in-order.
            wsrc = consts.tile([K, 512], mybir.dt.bfloat16)
            nc.gpsimd.memset(wsrc[:], 0)
            warm = psumS[0].tile([128, SPLITS[0][1]], f32, name="ptS", tag="ptS")
            for _ in range(5):
                nc.tensor.matmul(
                    warm[:, 0:512], wsrc[:, 0:128], wsrc[:, 0:512], start=True, stop=True
                )
            warm_tile = warm

            # cols 0..63: DVE r-min (host adds a^2); 64..127: staged d^2 min
            mins_sb = outp.tile([128, 4 * M_TILES], f32)
            trash_b = outp.tile([128, max(s for _, s in SPLITS)], bf16)

            for m in range(M_TILES):
                lw = inp_sb[:, m * 128 : (m + 1) * 128]
                for h in range(2):
                    col = 2 * m + h
                    phase = col % 2
                    r_w, s_w = SPLITS[phase]
                    ptR = psumR[phase].tile([128, r_w], f32)
                    if warm_tile is not None and phase == 0:
                        ptS, warm_tile = warm_tile, None
                    else:
                        ptS = psumS[phase].tile([128, s_w], f32, name="ptS", tag="ptS")
                    j0 = N + h * J_HALF
                    for q in range(r_w // 512):
                        nc.tensor.matmul(
                            ptR[:, q * 512 : (q + 1) * 512],
                            lw,
                            inp_sb[:, j0 + q * 512 : j0 + (q + 1) * 512],
                            start=True,
                            stop=True,
                        )
                    for q in range(s_w // 512):
                        nc.tensor.matmul(
                            ptS[:, q * 512 : (q + 1) * 512],
                            lw,
                            inp_sb[:, j0 + r_w + q * 512 : j0 + r_w + (q + 1) * 512],
                            start=True,
                            stop=True,
                        )
                    st = stage.tile([128, s_w], bf16)
                    nc.scalar.activation(
                        st[:],
                        ptS[:],
                        mybir.ActivationFunctionType.Identity,
                        bias=a2_sb[:, m : m + 1],
                        scale=1.0,
                    )
                    nc.vector.tensor_reduce(
                        mins_sb[:, col : col + 1],
                        ptR[:],
                        axis=mybir.AxisListType.X,
                        op=amin,
                    )
                    nc.vector.tensor_scalar(
                        out=trash_b[:, 0:s_w],
                        in0=st[:],
                        scalar1=FMAX,
                        scalar2=None,
                        op0=amin,
                        op1=amin,
                        accum_out=mins_sb[:, 64 + col : 65 + col],
                    )
                    if col == 47:
                        # Early partial output DMA overlaps the write-back
                        # latency with the remaining chunks.
                        nc.sync.dma_start(mins_ap[:, 0:48], mins_sb[:, 0:48])
                        nc.sync.dma_start(mins_ap[:, 64:112], mins_sb[:, 64:112])
            nc.sync.dma_start(mins_ap[:, 48:64], mins_sb[:, 48:64])
            nc.sync.dma_start(mins_ap[:, 112:128], mins_sb[:, 112:128])

    _split_multi_waits(nc)
    _NC_CACHE = nc
    return nc


def _split3(x):
    """x (f32) -> three bf16 parts whose (f32) sum ~= x to ~2^-27 rel."""
    x = x.astype(np.float32)
    h = x.astype(BF16).astype(np.float32)
    r = x - h
    l = r.astype(BF16).astype(np.float32)
    q = (r - l).astype(BF16).astype(np.float32)
    return h, l, q


def _prep_problem(A, B):
    """Build lhsT [K, N] and rhs [K, N] bf16 rows for r = |b|^2 - 2 a.b."""
    b2 = (B.astype(np.float64) ** 2).sum(1).astype(np.float32)
    b2h, b2l, b2q = _split3(b2)
    ah, al, aq = _split3(A)
    bh, bl, bq = _split3(B)
    ones = np.ones(N, np.float32)
    lhs_rows = [ones, ones, ones]
    rhs_rows = [b2h, b2l, b2q]
    for d in range(3):
        for a_, b_ in (
            (ah[:, d], -2.0 * bh[:, d]),
            (ah[:, d], -2.0 * bl[:, d]),
            (al[:, d], -2.0 * bh[:, d]),
            (al[:, d], -2.0 * bl[:, d]),
            (ah[:, d], -2.0 * bq[:, d]),
            (aq[:, d], -2.0 * bh[:, d]),
        ):
            lhs_rows.append(a_)
            rhs_rows.append(b_)
    lhsT = np.stack(lhs_rows).astype(BF16)
    rhs = np.stack(rhs_rows).astype(BF16)
    return np.concatenate([lhsT, rhs], axis=1)  # [K, 2N]


def _run(data1, data2, trace=False):
    d1 = np.asarray(data1, dtype=np.float32).reshape(8, N, 3)
    d2 = np.asarray(data2, dtype=np.float32).reshape(8, N, 3)
    in_maps = []
    a2_all = []
    for p in range(8):
        a2 = (d1[p].astype(np.float64) ** 2).sum(1).astype(np.float32)
        a2_all.append(a2)
        in_maps.append(
            {
                "inp": _prep_problem(d1[p], d2[p]),
                "a2": np.ascontiguousarray(a2.reshape(M_TILES, 128).T),
            }
        )
    nc = _build_nc()
    res = run_bass_kernel_spmd(nc, in_maps, core_ids=list(range(8)), trace=trace)

    out = np.zeros(2, np.float64)
    for p in range(8):
        m = res.results[p]["mins"]          # [128, 128]
        a2c = in_maps[p]["a2"]              # [128, 32]
        mr = m[:, :64].reshape(128, M_TILES, 2).min(axis=-1) + a2c   # DVE share
        ms = m[:, 64:].reshape(128, M_TILES, 2).min(axis=-1)         # staged share
        mflat = np.minimum(mr, ms).T.reshape(N).astype(np.float64)
        dd = np.sqrt(np.maximum(mflat, 0.0))
        out[p // 4] += dd.mean() / 4.0
    return out.astype(np.float32), res


def kernel(data1, data2, dim):
    dim = int(dim)
    if dim > 0:
        data1 = np.swapaxes(np.asarray(data1), 0, dim)
        data2 = np.swapaxes(np.asarray(data2), 0, dim)
    out, _ = _run(data1, data2, trace=False)
    return out


def kernel_traced(data1, data2, dim):
    """test.py entry: returns (output, BassKernelResults) with profiling."""
    dim = int(dim)
    if dim > 0:
        data1 = np.swapaxes(np.asarray(data1), 0, dim)
        data2 = np.swapaxes(np.asarray(data2), 0, dim)
    return _run(data1, data2, trace=True)
